# revision 28
# baseline (speedup 1.0000x reference)
"""Causal attention kernel for 8 TRN2 NeuronCores (Bass/Tile).

Problem: x [B=4, N=2048, Din=1024] f32, W_{q,k,v} [Dout=1024, Din] f32.
  q/k/v = x @ W.T ; S = q @ k.T (causal masked) ; P = softmax(S/sqrt(Dout)) ;
  out = P @ v.

Sharding: 8 cores = 4 batches x 2 "halves". Each core handles 1024 query
rows of one batch. Core (b, m=0) takes q rows [0:512)+[1536:2048), core
(b, m=1) takes [512:1536) -- this balances the causal-attention area while
keeping chunk widths uniform. Every core projects K/V for the full 2048-row
sequence of its batch (duplicated across the pair) so cores are independent
(no collectives).

Device program (SPMD, identical on all cores; per-core behavior comes only
from the data: which x columns are its queries, and the causal mask tiles):
  Phase A: Q^T [Dout, R], K^T [Dout, N], V [N, Dout] projections from
    host-pretransposed x^T / W^T, spilled to internal DRAM.
  Phase B: per 512-wide query chunk: S^T tiles [128k, 512q] = K^T.T @ Q^T,
    causal mask via copy_predicated(-1e30), P^T = exp(scale*S^T) (no
    max-subtraction -- scores are bounded, exp is safe in f32), then
    O[q,:] = P.T.T @ [V] with an extra ones-column matmul accumulating the
    softmax denominator, and a final per-row reciprocal scale.

Numerically everything is fp32 (PE fp32 mode, fp32 PSUM accumulate).
"""

import math
import os
from contextlib import ExitStack
from dataclasses import dataclass, field

import numpy as np

import concourse.bass as bass
import concourse.mybir as mybir
import concourse.tile as tile
from concourse import bacc
from concourse.bass_utils import run_bass_kernel_spmd

P = 128
F32 = mybir.dt.float32
F32R = mybir.dt.float32r
U8 = mybir.dt.uint8
NEG = -1.0e30
# matmul operand dtype: float32r runs the PE at 4x fp32 throughput for
# N>=256 at ~tf32 precision (measured 1.4e-4 rel on d=1024 contractions)
MM = F32R


@dataclass(frozen=True)
class Cfg:
    SEQ: int          # kv sequence length per batch
    D: int            # Din == Dout
    R: int            # query rows handled per core
    CW: int           # chunk width (<= 512)
    st_ext: tuple     # per chunk: number of k-tiles to compute S^T/P^T for
    av_ext: tuple     # per chunk, per 128-block: k-tiles to accumulate in AV
    pairs: int = 4    # unused (kept for cfg compatibility)

    @property
    def DT(self):  # contraction tiles
        return self.D // P

    @property
    def T(self):   # kv tiles
        return self.SEQ // P

    @property
    def NCH(self):  # query chunks per core
        return self.R // self.CW

    @property
    def OCH(self):  # output-column chunks (N<=512 per matmul)
        return max(1, self.D // 512)

    @property
    def OCW(self):
        return self.D // self.OCH

    @property
    def n_mask_tiles(self):
        return sum(self.st_ext)

    @property
    def scale(self):
        return 1.0 / math.sqrt(self.D)


def real_cfg():
    return Cfg(
        SEQ=2048, D=1024, R=1024, CW=512,
        st_ext=(8, 16),
        av_ext=((2, 4, 6, 8), (10, 12, 14, 16)),
    )


# q-block (128-row) assignment per core half m
def q_blocks(cfg: Cfg, m: int):
    nb_total = cfg.SEQ // P
    return list(range(m, nb_total, 2))


def _emit(ctx: ExitStack, tc: tile.TileContext, cfg: Cfg, aps):
    nc = tc.nc
    DT, T, CW, NCH, D, SEQ = cfg.DT, cfg.T, cfg.CW, cfg.NCH, cfg.D, cfg.SEQ
    OCH, OCW = cfg.OCH, cfg.OCW
    KCH = SEQ // CW  # kv chunks for projections
    KB = CW // P     # 128-blocks per chunk

    xT, xTq, wqT, wkT, wvT, mask, o_ap = (
        aps["xT"], aps["xTq"], aps["wqT"], aps["wkT"], aps["wvT"],
        aps["mask"], aps["o"],
    )

    dram = ctx.enter_context(tc.tile_pool(name="dram", bufs=1, space="DRAM"))
    qT_ds = [dram.tile([D, CW], MM, name=f"qTd{c}") for c in range(NCH)]
    kT_ds = [dram.tile([D, CW], MM, name=f"kTd{c}") for c in range(KCH)]

    OH = 2
    OB = DT // OH
    D2 = D // OH
    NST = min(2, DT // 2) or 1  # spill-write batch (o-tiles per DMA)

    wpool = ctx.enter_context(tc.tile_pool(name="wres", bufs=3))
    xpool = ctx.enter_context(tc.tile_pool(name="xstream", bufs=2))
    stage = ctx.enter_context(tc.tile_pool(name="stageA", bufs=2))
    ps512 = ctx.enter_context(tc.tile_pool(name="ps512", bufs=3, space="PSUM"))
    psO = ctx.enter_context(tc.tile_pool(name="psO", bufs=2, space="PSUM"))
    psD = ctx.enter_context(tc.tile_pool(name="psD", bufs=1, space="PSUM"))
    cpool = ctx.enter_context(tc.tile_pool(name="consts", bufs=1))
    vpool = ctx.enter_context(tc.tile_pool(name="vres", bufs=1))
    qpool = ctx.enter_context(tc.tile_pool(name="qc", bufs=1))
    ppool = ctx.enter_context(tc.tile_pool(name="pT", bufs=16))
    kpool = ctx.enter_context(tc.tile_pool(name="kt", bufs=4))
    mpool = ctx.enter_context(tc.tile_pool(name="mt", bufs=2))
    spool = ctx.enter_context(tc.tile_pool(name="stageB", bufs=1))

    ones_f = cpool.tile([P, 1], F32, tag="ones_f")
    nc.vector.memset(ones_f, 1.0)
    v_sb = vpool.tile([P, T, D], MM, tag="v")

    def w_ap(wT, h):
        return wT.rearrange("(dt p) o -> p dt o", p=P)[:, :, h * D2:(h + 1) * D2]

    def x_load(xap, c, name):
        xs = xpool.tile([P, DT, CW], MM, tag="xs", name=name)
        rr = xap.rearrange("(dt p) n -> p dt n", p=P)[:, :, c * CW:(c + 1) * CW]
        hd = DT // 2
        nc.sync.dma_start(xs[:, :hd, :], rr[:, :hd, :])
        nc.sync.dma_start(xs[:, hd:, :], rr[:, hd:, :])
        return xs

    def w_load(wT, h, name):
        wh = wpool.tile([P, DT, D2], MM, tag="w", name=name)
        half = D2 // 2
        rr = wT.rearrange("(dt p) o -> p dt o", p=P)
        nc.sync.dma_start(wh[:, :, :half], rr[:, :, h * D2:h * D2 + half])
        nc.sync.dma_start(wh[:, :, half:], rr[:, :, h * D2 + half:(h + 1) * D2])
        return wh

    # ---------------- Phase A: projections ----------------
    # Q^T -> qT_ds[c]
    # emission order tuned for the in-order DMA queue: first W half, then
    # the first x chunk, then the remaining W halves and prefetched x
    wqs = []
    rrq = wqT.rearrange("(dt p) o -> p dt o", p=P)
    for h in range(OH):
        wqs.append(wpool.tile([P, DT, D2], MM, tag="w", name=f"wq{h}"))
    half = D2 // 2
    nc.sync.dma_start(wqs[0][:, :, :half], rrq[:, :, :half])
    xqs = []
    for c in range(NCH):
        xqs.append(xpool.tile([P, DT, CW], MM, tag="xs", name=f"xq{c}"))
        rr = xTq.rearrange("(dt p) n -> p dt n", p=P)[:, :, c * CW:(c + 1) * CW]
        hd = DT // 2
        nc.sync.dma_start(xqs[c][:, :hd, :], rr[:, :hd, :])
        nc.sync.dma_start(xqs[c][:, hd:, :], rr[:, hd:, :])
        if c == 0:
            nc.sync.dma_start(wqs[0][:, :, half:], rrq[:, :, half:D2])
            for h in range(1, OH):
                nc.sync.dma_start(wqs[h], rrq[:, :, h * D2:(h + 1) * D2])
    for c in range(NCH):
        xq = xqs[c]
        for h in range(OH):
            for og in range(OB // NST):
                st = stage.tile([P, NST, CW], MM, tag="stA", name="stq")
                for j in range(NST):
                    o4 = og * NST + j
                    ps = ps512.tile([P, CW], F32, tag="ps512", name="psq")
                    for dt in range(DT):
                        nc.tensor.matmul(
                            ps, wqs[h][:, dt, o4 * P:(o4 + 1) * P], xq[:, dt, :],
                            start=(dt == 0), stop=(dt == DT - 1))
                    nc.vector.tensor_copy(st[:, j, :], ps)
                o0 = h * OB + og * NST
                nc.sync.dma_start(
                    qT_ds[c].rearrange("(o p) w -> p o w", p=P)[:, o0:o0 + NST, :],
                    st)

    # K^T -> kT_ds[c]
    wks = None
    for c in range(KCH):
        xk = x_load(xT, c, f"xk{c}")
        if wks is None:
            wks = [w_load(wkT, h, f"wk{h}") for h in range(OH)]
        for h in range(OH):
            for og in range(OB // NST):
                st = stage.tile([P, NST, CW], MM, tag="stA", name="stk")
                for j in range(NST):
                    o4 = og * NST + j
                    ps = ps512.tile([P, CW], F32, tag="ps512", name="psk")
                    for dt in range(DT):
                        nc.tensor.matmul(
                            ps, wks[h][:, dt, o4 * P:(o4 + 1) * P], xk[:, dt, :],
                            start=(dt == 0), stop=(dt == DT - 1))
                    nc.vector.tensor_copy(st[:, j, :], ps)
                o0 = h * OB + og * NST
                nc.sync.dma_start(
                    kT_ds[c].rearrange("(o p) w -> p o w", p=P)[:, o0:o0 + NST, :],
                    st)

    # V -> resident v_sb; x read once, both wv halves held
    wvs = None
    for c in range(KCH):
        xk = x_load(xT, c, f"xkv{c}")
        if wvs is None:
            wvs = []
            for oc in range(OCH):
                wvh = wpool.tile([P, DT, OCW], MM, tag="w", name=f"wv{oc}")
                nc.sync.dma_start(
                    wvh,
                    wvT.rearrange("(dt p) o -> p dt o", p=P)
                    [:, :, oc * OCW:(oc + 1) * OCW])
                wvs.append(wvh)
        for kb in range(KB):
            for oc in range(OCH):
                ps = ps512.tile([P, OCW], F32, tag="ps512", name="psv")
                for dt in range(DT):
                    nc.tensor.matmul(
                        ps, xk[:, dt, kb * P:(kb + 1) * P], wvs[oc][:, dt, :],
                        start=(dt == 0), stop=(dt == DT - 1))
                nc.vector.tensor_copy(
                    v_sb[:, c * KB + kb, oc * OCW:(oc + 1) * OCW], ps)

    # ---------------- Phase B: attention ----------------
    mask_idx = 0
    for c in range(NCH):
        qc = qpool.tile([P, DT, CW], MM, tag="qc", name=f"qc{c}")
        nc.scalar.dma_start(qc, qT_ds[c].rearrange("(dt p) w -> p dt w", p=P))

        pTs = []
        for t in range(cfg.st_ext[c]):
            kt = kpool.tile([P, DT, P], MM, tag="kt")
            kc, ko = t // KB, (t % KB) * P
            nc.sync.dma_start(
                kt, kT_ds[kc].rearrange("(dt p) w -> p dt w", p=P)[:, :, ko:ko + P])
            ps = ps512.tile([P, CW], F32, tag="ps512", name="pss")
            for dt in range(DT):
                nc.tensor.matmul(
                    ps, kt[:, dt, :], qc[:, dt, :],
                    start=(dt == 0), stop=(dt == DT - 1))
            pT = ppool.tile([P, CW], MM, tag="pT", name=f"pT_{c}_{t}")
            nc.scalar.activation(
                pT, ps, mybir.ActivationFunctionType.Exp, scale=cfg.scale)
            mt = mpool.tile([P, CW], U8, tag="mt")
            nc.sync.dma_start(mt, mask[mask_idx])
            mask_idx += 1
            nc.vector.tensor_mul(pT, pT, mt)
            pTs.append(pT)

        psd = psD.tile([P, KB], F32, tag="psD", name=f"psd{c}")
        for b in range(KB):
            pso = psO.tile([P, D], F32, tag="psO")
            E = cfg.av_ext[c][b]
            for t in range(E):
                lh = pTs[t][:, b * P:(b + 1) * P]
                for oc in range(OCH):
                    nc.tensor.matmul(
                        pso[:, oc * OCW:(oc + 1) * OCW], lh,
                        v_sb[:, t, oc * OCW:(oc + 1) * OCW],
                        start=(t == 0), stop=(t == E - 1))
            for t in range(E):
                lh = pTs[t][:, b * P:(b + 1) * P]
                nc.tensor.matmul(
                    psd[:, b:b + 1], lh.bitcast(F32), ones_f,
                    start=(t == 0), stop=(t == E - 1))
            rcp = spool.tile([P, 1], F32, tag="rcp")
            nc.vector.reciprocal(rcp, psd[:, b:b + 1])
            osb = spool.tile([P, D], F32, tag="osb")
            nc.scalar.activation(
                osb, pso, mybir.ActivationFunctionType.Copy,
                scale=rcp[:, 0:1])
            nc.scalar.dma_start(o_ap[c * CW + b * P:c * CW + (b + 1) * P, :], osb)


def build_program(cfg: Cfg):
    nc = bacc.Bacc("TRN2", dynamic_dma_scratch_size=2048)
    aps = {
        "xT": nc.dram_tensor("xT", [cfg.D, cfg.SEQ], MM, kind="ExternalInput").ap(),
        "xTq": nc.dram_tensor("xTq", [cfg.D, cfg.R], MM, kind="ExternalInput").ap(),
        "wqT": nc.dram_tensor("wqT", [cfg.D, cfg.D], MM, kind="ExternalInput").ap(),
        "wkT": nc.dram_tensor("wkT", [cfg.D, cfg.D], MM, kind="ExternalInput").ap(),
        "wvT": nc.dram_tensor("wvT", [cfg.D, cfg.D], MM, kind="ExternalInput").ap(),
        "mask": nc.dram_tensor(
            "mask", [cfg.n_mask_tiles, P, cfg.CW], U8, kind="ExternalInput").ap(),
        "o": nc.dram_tensor("o", [cfg.R, cfg.D], F32, kind="ExternalOutput").ap(),
    }
    with tile.TileContext(nc) as tc:
        with ExitStack() as ctx:
            _emit(ctx, tc, cfg, aps)
    nc.compile()
    return nc


def make_mask(cfg: Cfg, qglob: np.ndarray) -> np.ndarray:
    """u8 mask tiles: 1 where k_global > q_global (entry masked out)."""
    m = np.zeros((cfg.n_mask_tiles, P, cfg.CW), dtype=np.uint8)
    idx = 0
    for c in range(cfg.NCH):
        qg = qglob[c * cfg.CW:(c + 1) * cfg.CW]  # [CW]
        for t in range(cfg.st_ext[c]):
            kg = np.arange(t * P, (t + 1) * P)  # [P]
            m[idx] = (kg[:, None] <= qg[None, :]).astype(np.uint8)
            idx += 1
    return m


def make_core_inputs(cfg: Cfg, x_b: np.ndarray, wqT, wkT, wvT, m: int):
    blocks = q_blocks(cfg, m)
    qglob = np.concatenate([np.arange(b * P, (b + 1) * P) for b in blocks])
    xT_b = np.ascontiguousarray(x_b.T)
    return {
        "xT": xT_b,
        "xTq": np.ascontiguousarray(xT_b[:, qglob]),
        "wqT": wqT,
        "wkT": wkT,
        "wvT": wvT,
        "mask": make_mask(cfg, qglob),
    }, qglob


_prog_cache = {}


def get_program(cfg: Cfg):
    if cfg not in _prog_cache:
        _prog_cache[cfg] = build_program(cfg)
    return _prog_cache[cfg]


def run(x, W_query, W_key, W_value, trace=False, trace_cores=None):
    """Returns (out [B, N, D], BassKernelResults)."""
    cfg = real_cfg()
    B = x.shape[0]
    nc = get_program(cfg)
    wqT = np.ascontiguousarray(np.asarray(W_query, dtype=np.float32).T)
    wkT = np.ascontiguousarray(np.asarray(W_key, dtype=np.float32).T)
    wvT = np.ascontiguousarray(np.asarray(W_value, dtype=np.float32).T)
    x = np.asarray(x, dtype=np.float32)

    in_maps = []
    qglobs = []
    for core in range(2 * B):
        b, m = core // 2, core % 2
        im, qglob = make_core_inputs(cfg, x[b], wqT, wkT, wvT, m)
        in_maps.append(im)
        qglobs.append(qglob)

    res = run_bass_kernel_spmd(
        nc, in_maps, list(range(2 * B)), trace=trace,
        trace_cores=trace_cores)

    out = np.empty((B, cfg.SEQ, cfg.D), dtype=np.float32)
    for core in range(2 * B):
        b = core // 2
        out[b][qglobs[core]] = res.results[core]["o"]
    return out, res


def kernel(**inputs) -> np.ndarray:
    out, _ = run(
        inputs["x"], inputs["W_query"], inputs["W_key"], inputs["W_value"])
    return out


# revision 29
# speedup vs baseline: 1.0788x; 1.0788x over previous
"""Causal attention kernel for 8 TRN2 NeuronCores (Bass/Tile).

Problem: x [B=4, N=2048, Din=1024] f32, W_{q,k,v} [Dout=1024, Din] f32.
  q/k/v = x @ W.T ; S = q @ k.T (causal masked) ; P = softmax(S/sqrt(Dout)) ;
  out = P @ v.

Sharding: 8 cores = 4 batches x 2 "halves". Each core handles 1024 query
rows of one batch. Core (b, m=0) takes q rows [0:512)+[1536:2048), core
(b, m=1) takes [512:1536) -- this balances the causal-attention area while
keeping chunk widths uniform. Every core projects K/V for the full 2048-row
sequence of its batch (duplicated across the pair) so cores are independent
(no collectives).

Device program (SPMD, identical on all cores; per-core behavior comes only
from the data: which x columns are its queries, and the causal mask tiles):
  Phase A: Q^T [Dout, R], K^T [Dout, N], V [N, Dout] projections from
    host-pretransposed x^T / W^T, spilled to internal DRAM.
  Phase B: per 512-wide query chunk: S^T tiles [128k, 512q] = K^T.T @ Q^T,
    causal mask via copy_predicated(-1e30), P^T = exp(scale*S^T) (no
    max-subtraction -- scores are bounded, exp is safe in f32), then
    O[q,:] = P.T.T @ [V] with an extra ones-column matmul accumulating the
    softmax denominator, and a final per-row reciprocal scale.

Numerically everything is fp32 (PE fp32 mode, fp32 PSUM accumulate).
"""

import math
import os
from contextlib import ExitStack
from dataclasses import dataclass, field

import numpy as np

import concourse.bass as bass
import concourse.mybir as mybir
import concourse.tile as tile
from concourse import bacc
from concourse.bass_utils import run_bass_kernel_spmd

P = 128
F32 = mybir.dt.float32
F32R = mybir.dt.float32r
U8 = mybir.dt.uint8
NEG = -1.0e30
# matmul operand dtype: float32r runs the PE at 4x fp32 throughput for
# N>=256 at ~tf32 precision (measured 1.4e-4 rel on d=1024 contractions)
MM = F32R


@dataclass(frozen=True)
class Cfg:
    SEQ: int          # kv sequence length per batch
    D: int            # Din == Dout
    R: int            # query rows handled per core
    CW: int           # chunk width (<= 512)
    st_ext: tuple     # per chunk: number of k-tiles to compute S^T/P^T for
    av_ext: tuple     # per chunk, per 128-block: k-tiles to accumulate in AV
    pairs: int = 4    # unused (kept for cfg compatibility)

    @property
    def DT(self):  # contraction tiles
        return self.D // P

    @property
    def T(self):   # kv tiles
        return self.SEQ // P

    @property
    def NCH(self):  # query chunks per core
        return self.R // self.CW

    @property
    def OCH(self):  # output-column chunks (N<=512 per matmul)
        return max(1, self.D // 512)

    @property
    def OCW(self):
        return self.D // self.OCH

    @property
    def n_mask_tiles(self):
        return sum(self.st_ext)

    @property
    def scale(self):
        return 1.0 / math.sqrt(self.D)


def real_cfg():
    return Cfg(
        SEQ=2048, D=1024, R=1024, CW=512,
        st_ext=(8, 16),
        av_ext=((2, 4, 6, 8), (10, 12, 14, 16)),
    )


# q-block (128-row) assignment per core half m
def q_blocks(cfg: Cfg, m: int):
    nb_total = cfg.SEQ // P
    return list(range(m, nb_total, 2))


def _emit(ctx: ExitStack, tc: tile.TileContext, cfg: Cfg, aps):
    nc = tc.nc
    DT, T, CW, NCH, D, SEQ = cfg.DT, cfg.T, cfg.CW, cfg.NCH, cfg.D, cfg.SEQ
    OCH, OCW = cfg.OCH, cfg.OCW
    KCH = SEQ // CW  # kv chunks for projections
    KB = CW // P     # 128-blocks per chunk

    xT, xTq, wqT, wkT, wvT, mask, o_ap = (
        aps["xT"], aps["xTq"], aps["wqT"], aps["wkT"], aps["wvT"],
        aps["mask"], aps["o"],
    )

    dram = ctx.enter_context(tc.tile_pool(name="dram", bufs=1, space="DRAM"))
    qT_ds = [dram.tile([D, CW], MM, name=f"qTd{c}") for c in range(NCH)]
    kT_ds = [dram.tile([D, CW], MM, name=f"kTd{c}") for c in range(KCH)]

    OH = 2
    OB = DT // OH
    D2 = D // OH
    NST = min(2, DT // 2) or 1  # spill-write batch (o-tiles per DMA)

    wpool = ctx.enter_context(tc.tile_pool(name="wres", bufs=3))
    xpool = ctx.enter_context(tc.tile_pool(name="xstream", bufs=2))
    stage = ctx.enter_context(tc.tile_pool(name="stageA", bufs=2))
    ps512 = ctx.enter_context(tc.tile_pool(name="ps512", bufs=3, space="PSUM"))
    psO = ctx.enter_context(tc.tile_pool(name="psO", bufs=2, space="PSUM"))
    psD = ctx.enter_context(tc.tile_pool(name="psD", bufs=1, space="PSUM"))
    cpool = ctx.enter_context(tc.tile_pool(name="consts", bufs=1))
    vpool = ctx.enter_context(tc.tile_pool(name="vres", bufs=1))
    qpool = ctx.enter_context(tc.tile_pool(name="qc", bufs=1))
    ppool = ctx.enter_context(tc.tile_pool(name="pT", bufs=16))
    kpool = ctx.enter_context(tc.tile_pool(name="kt", bufs=3))
    mpool = ctx.enter_context(tc.tile_pool(name="mt", bufs=2))
    spool = ctx.enter_context(tc.tile_pool(name="stageB", bufs=1))

    ones_f = cpool.tile([P, 1], F32, tag="ones_f")
    nc.vector.memset(ones_f, 1.0)
    v_sb = vpool.tile([P, T, D], MM, tag="v")

    def w_ap(wT, h):
        return wT.rearrange("(dt p) o -> p dt o", p=P)[:, :, h * D2:(h + 1) * D2]

    def x_load(xap, c, name):
        xs = xpool.tile([P, DT, CW], MM, tag="xs", name=name)
        rr = xap.rearrange("(dt p) n -> p dt n", p=P)[:, :, c * CW:(c + 1) * CW]
        hd = DT // 2
        nc.sync.dma_start(xs[:, :hd, :], rr[:, :hd, :])
        nc.sync.dma_start(xs[:, hd:, :], rr[:, hd:, :])
        return xs

    def w_load(wT, h, name):
        wh = wpool.tile([P, DT, D2], MM, tag="w", name=name)
        half = D2 // 2
        rr = wT.rearrange("(dt p) o -> p dt o", p=P)
        nc.sync.dma_start(wh[:, :, :half], rr[:, :, h * D2:h * D2 + half])
        nc.sync.dma_start(wh[:, :, half:], rr[:, :, h * D2 + half:(h + 1) * D2])
        return wh

    # ---------------- Phase A: projections ----------------
    # Q^T -> qT_ds[c]
    # emission order tuned for the in-order DMA queue: first W half, then
    # the first x chunk, then the remaining W halves and prefetched x
    wqs = []
    rrq = wqT.rearrange("(dt p) o -> p dt o", p=P)
    for h in range(OH):
        wqs.append(wpool.tile([P, DT, D2], MM, tag="w", name=f"wq{h}"))
    half = D2 // 2
    nc.sync.dma_start(wqs[0][:, :, :half], rrq[:, :, :half])
    xqs = []
    for c in range(NCH):
        xqs.append(xpool.tile([P, DT, CW], MM, tag="xs", name=f"xq{c}"))
        rr = xTq.rearrange("(dt p) n -> p dt n", p=P)[:, :, c * CW:(c + 1) * CW]
        hd = DT // 2
        nc.sync.dma_start(xqs[c][:, :hd, :], rr[:, :hd, :])
        nc.sync.dma_start(xqs[c][:, hd:, :], rr[:, hd:, :])
        if c == 0:
            nc.sync.dma_start(wqs[0][:, :, half:], rrq[:, :, half:D2])
            for h in range(1, OH):
                nc.sync.dma_start(wqs[h], rrq[:, :, h * D2:(h + 1) * D2])
    for c in range(NCH):
        xq = xqs[c]
        for h in range(OH):
            for og in range(OB // NST):
                st = stage.tile([P, NST, CW], MM, tag="stA", name="stq")
                for j in range(NST):
                    o4 = og * NST + j
                    ps = ps512.tile([P, CW], F32, tag="ps512", name="psq")
                    for dt in range(DT):
                        nc.tensor.matmul(
                            ps, wqs[h][:, dt, o4 * P:(o4 + 1) * P], xq[:, dt, :],
                            start=(dt == 0), stop=(dt == DT - 1))
                    nc.vector.tensor_copy(st[:, j, :], ps)
                o0 = h * OB + og * NST
                nc.sync.dma_start(
                    qT_ds[c].rearrange("(o p) w -> p o w", p=P)[:, o0:o0 + NST, :],
                    st)

    # K^T -> kT_ds[c]
    wks = None
    for c in range(KCH):
        xk = x_load(xT, c, f"xk{c}")
        if wks is None:
            wks = [w_load(wkT, h, f"wk{h}") for h in range(OH)]
        for h in range(OH):
            for og in range(OB // NST):
                st = stage.tile([P, NST, CW], MM, tag="stA", name="stk")
                for j in range(NST):
                    o4 = og * NST + j
                    ps = ps512.tile([P, CW], F32, tag="ps512", name="psk")
                    for dt in range(DT):
                        nc.tensor.matmul(
                            ps, wks[h][:, dt, o4 * P:(o4 + 1) * P], xk[:, dt, :],
                            start=(dt == 0), stop=(dt == DT - 1))
                    nc.vector.tensor_copy(st[:, j, :], ps)
                o0 = h * OB + og * NST
                nc.sync.dma_start(
                    kT_ds[c].rearrange("(o p) w -> p o w", p=P)[:, o0:o0 + NST, :],
                    st)

    # V -> resident v_sb; x read once, both wv halves held
    wvs = None
    for c in range(KCH):
        xk = x_load(xT, c, f"xkv{c}")
        if wvs is None:
            wvs = []
            for oc in range(OCH):
                wvh = wpool.tile([P, DT, OCW], MM, tag="w", name=f"wv{oc}")
                nc.sync.dma_start(
                    wvh,
                    wvT.rearrange("(dt p) o -> p dt o", p=P)
                    [:, :, oc * OCW:(oc + 1) * OCW])
                wvs.append(wvh)
        for kb in range(KB):
            for oc in range(OCH):
                ps = ps512.tile([P, OCW], F32, tag="ps512", name="psv")
                for dt in range(DT):
                    nc.tensor.matmul(
                        ps, xk[:, dt, kb * P:(kb + 1) * P], wvs[oc][:, dt, :],
                        start=(dt == 0), stop=(dt == DT - 1))
                nc.vector.tensor_copy(
                    v_sb[:, c * KB + kb, oc * OCW:(oc + 1) * OCW], ps)

    # ---------------- Phase B: attention ----------------
    mask_idx = 0
    for c in range(NCH):
        qc = qpool.tile([P, DT, CW], MM, tag="qc", name=f"qc{c}")
        nc.scalar.dma_start(qc, qT_ds[c].rearrange("(dt p) w -> p dt w", p=P))

        pTs = []
        for t in range(cfg.st_ext[c]):
            kt = kpool.tile([P, DT, P], MM, tag="kt")
            kc, ko = t // KB, (t % KB) * P
            nc.sync.dma_start(
                kt, kT_ds[kc].rearrange("(dt p) w -> p dt w", p=P)[:, :, ko:ko + P])
            ps = ps512.tile([P, CW], F32, tag="ps512", name="pss")
            for dt in range(DT):
                nc.tensor.matmul(
                    ps, kt[:, dt, :], qc[:, dt, :],
                    start=(dt == 0), stop=(dt == DT - 1))
            pT = ppool.tile([P, CW], MM, tag="pT", name=f"pT_{c}_{t}")
            nc.scalar.activation(
                pT, ps, mybir.ActivationFunctionType.Exp, scale=cfg.scale)
            mt = mpool.tile([P, CW], U8, tag="mt")
            nc.sync.dma_start(mt, mask[mask_idx])
            mask_idx += 1
            nc.vector.tensor_mul(pT, pT, mt)
            pTs.append(pT)

        psd = psD.tile([P, KB], F32, tag="psD", name=f"psd{c}")
        for b in range(KB):
            pso = psO.tile([P, D], F32, tag="psO")
            E = cfg.av_ext[c][b]
            for t in range(E):
                lh = pTs[t][:, b * P:(b + 1) * P]
                for oc in range(OCH):
                    nc.tensor.matmul(
                        pso[:, oc * OCW:(oc + 1) * OCW], lh,
                        v_sb[:, t, oc * OCW:(oc + 1) * OCW],
                        start=(t == 0), stop=(t == E - 1))
            for t in range(E):
                lh = pTs[t][:, b * P:(b + 1) * P]
                nc.tensor.matmul(
                    psd[:, b:b + 1], lh.bitcast(F32), ones_f,
                    start=(t == 0), stop=(t == E - 1))
            rcp = spool.tile([P, 1], F32, tag="rcp")
            nc.vector.reciprocal(rcp, psd[:, b:b + 1])
            osb = spool.tile([P, D], F32, tag="osb")
            nc.scalar.activation(
                osb, pso, mybir.ActivationFunctionType.Copy,
                scale=rcp[:, 0:1])
            nc.scalar.dma_start(o_ap[c * CW + b * P:c * CW + (b + 1) * P, :], osb)


def build_program(cfg: Cfg):
    nc = bacc.Bacc("TRN2", dynamic_dma_scratch_size=2048)
    aps = {
        "xT": nc.dram_tensor("xT", [cfg.D, cfg.SEQ], MM, kind="ExternalInput").ap(),
        "xTq": nc.dram_tensor("xTq", [cfg.D, cfg.R], MM, kind="ExternalInput").ap(),
        "wqT": nc.dram_tensor("wqT", [cfg.D, cfg.D], MM, kind="ExternalInput").ap(),
        "wkT": nc.dram_tensor("wkT", [cfg.D, cfg.D], MM, kind="ExternalInput").ap(),
        "wvT": nc.dram_tensor("wvT", [cfg.D, cfg.D], MM, kind="ExternalInput").ap(),
        "mask": nc.dram_tensor(
            "mask", [cfg.n_mask_tiles, P, cfg.CW], U8, kind="ExternalInput").ap(),
        "o": nc.dram_tensor("o", [cfg.R, cfg.D], F32, kind="ExternalOutput").ap(),
    }
    with tile.TileContext(nc) as tc:
        with ExitStack() as ctx:
            _emit(ctx, tc, cfg, aps)
    nc.compile()
    return nc


def make_mask(cfg: Cfg, qglob: np.ndarray) -> np.ndarray:
    """u8 mask tiles: 1 where k_global > q_global (entry masked out)."""
    m = np.zeros((cfg.n_mask_tiles, P, cfg.CW), dtype=np.uint8)
    idx = 0
    for c in range(cfg.NCH):
        qg = qglob[c * cfg.CW:(c + 1) * cfg.CW]  # [CW]
        for t in range(cfg.st_ext[c]):
            kg = np.arange(t * P, (t + 1) * P)  # [P]
            m[idx] = (kg[:, None] <= qg[None, :]).astype(np.uint8)
            idx += 1
    return m


def make_core_inputs(cfg: Cfg, x_b: np.ndarray, wqT, wkT, wvT, m: int):
    blocks = q_blocks(cfg, m)
    qglob = np.concatenate([np.arange(b * P, (b + 1) * P) for b in blocks])
    xT_b = np.ascontiguousarray(x_b.T)
    return {
        "xT": xT_b,
        "xTq": np.ascontiguousarray(xT_b[:, qglob]),
        "wqT": wqT,
        "wkT": wkT,
        "wvT": wvT,
        "mask": make_mask(cfg, qglob),
    }, qglob


_prog_cache = {}


def get_program(cfg: Cfg):
    if cfg not in _prog_cache:
        _prog_cache[cfg] = build_program(cfg)
    return _prog_cache[cfg]


def run(x, W_query, W_key, W_value, trace=False, trace_cores=None):
    """Returns (out [B, N, D], BassKernelResults)."""
    cfg = real_cfg()
    B = x.shape[0]
    nc = get_program(cfg)
    wqT = np.ascontiguousarray(np.asarray(W_query, dtype=np.float32).T)
    wkT = np.ascontiguousarray(np.asarray(W_key, dtype=np.float32).T)
    wvT = np.ascontiguousarray(np.asarray(W_value, dtype=np.float32).T)
    x = np.asarray(x, dtype=np.float32)

    in_maps = []
    qglobs = []
    for core in range(2 * B):
        b, m = core // 2, core % 2
        im, qglob = make_core_inputs(cfg, x[b], wqT, wkT, wvT, m)
        in_maps.append(im)
        qglobs.append(qglob)

    res = run_bass_kernel_spmd(
        nc, in_maps, list(range(2 * B)), trace=trace,
        trace_cores=trace_cores)

    out = np.empty((B, cfg.SEQ, cfg.D), dtype=np.float32)
    for core in range(2 * B):
        b = core // 2
        out[b][qglobs[core]] = res.results[core]["o"]
    return out, res


def kernel(**inputs) -> np.ndarray:
    out, _ = run(
        inputs["x"], inputs["W_query"], inputs["W_key"], inputs["W_value"])
    return out


# revision 30
# speedup vs baseline: 1.0799x; 1.0010x over previous
"""Causal attention kernel for 8 TRN2 NeuronCores (Bass/Tile).

Problem: x [B=4, N=2048, Din=1024] f32, W_{q,k,v} [Dout=1024, Din] f32.
  q/k/v = x @ W.T ; S = q @ k.T (causal masked) ; P = softmax(S/sqrt(Dout)) ;
  out = P @ v.

Sharding: 8 cores = 4 batches x 2 "halves". Each core handles 1024 query
rows of one batch. Core (b, m=0) takes q rows [0:512)+[1536:2048), core
(b, m=1) takes [512:1536) -- this balances the causal-attention area while
keeping chunk widths uniform. Every core projects K/V for the full 2048-row
sequence of its batch (duplicated across the pair) so cores are independent
(no collectives).

Device program (SPMD, identical on all cores; per-core behavior comes only
from the data: which x columns are its queries, and the causal mask tiles):
  Phase A: Q^T [Dout, R], K^T [Dout, N], V [N, Dout] projections from
    host-pretransposed x^T / W^T, spilled to internal DRAM.
  Phase B: per 512-wide query chunk: S^T tiles [128k, 512q] = K^T.T @ Q^T,
    causal mask via copy_predicated(-1e30), P^T = exp(scale*S^T) (no
    max-subtraction -- scores are bounded, exp is safe in f32), then
    O[q,:] = P.T.T @ [V] with an extra ones-column matmul accumulating the
    softmax denominator, and a final per-row reciprocal scale.

Numerically everything is fp32 (PE fp32 mode, fp32 PSUM accumulate).
"""

import math
from contextlib import ExitStack
from dataclasses import dataclass

import numpy as np

import concourse.bass as bass
import concourse.mybir as mybir
import concourse.tile as tile
from concourse import bacc
from concourse.bass_utils import run_bass_kernel_spmd

P = 128
F32 = mybir.dt.float32
F32R = mybir.dt.float32r
U8 = mybir.dt.uint8
NEG = -1.0e30
# matmul operand dtype: float32r runs the PE at 4x fp32 throughput for
# N>=256 at ~tf32 precision (measured 1.4e-4 rel on d=1024 contractions)
MM = F32R


@dataclass(frozen=True)
class Cfg:
    SEQ: int          # kv sequence length per batch
    D: int            # Din == Dout
    R: int            # query rows handled per core
    CW: int           # chunk width (<= 512)
    st_ext: tuple     # per chunk: number of k-tiles to compute S^T/P^T for
    av_ext: tuple     # per chunk, per 128-block: k-tiles to accumulate in AV
    pairs: int = 4    # unused (kept for cfg compatibility)

    @property
    def DT(self):  # contraction tiles
        return self.D // P

    @property
    def T(self):   # kv tiles
        return self.SEQ // P

    @property
    def NCH(self):  # query chunks per core
        return self.R // self.CW

    @property
    def OCH(self):  # output-column chunks (N<=512 per matmul)
        return max(1, self.D // 512)

    @property
    def OCW(self):
        return self.D // self.OCH

    @property
    def n_mask_tiles(self):
        return sum(self.st_ext)

    @property
    def scale(self):
        return 1.0 / math.sqrt(self.D)


def real_cfg():
    return Cfg(
        SEQ=2048, D=1024, R=1024, CW=512,
        st_ext=(8, 16),
        av_ext=((2, 4, 6, 8), (10, 12, 14, 16)),
    )


# q-block (128-row) assignment per core half m
def q_blocks(cfg: Cfg, m: int):
    nb_total = cfg.SEQ // P
    return list(range(m, nb_total, 2))


def _emit(ctx: ExitStack, tc: tile.TileContext, cfg: Cfg, aps):
    nc = tc.nc
    DT, T, CW, NCH, D, SEQ = cfg.DT, cfg.T, cfg.CW, cfg.NCH, cfg.D, cfg.SEQ
    OCH, OCW = cfg.OCH, cfg.OCW
    KCH = SEQ // CW  # kv chunks for projections
    KB = CW // P     # 128-blocks per chunk

    xT, xTq, wqT, wkT, wvT, mask, o_ap = (
        aps["xT"], aps["xTq"], aps["wqT"], aps["wkT"], aps["wvT"],
        aps["mask"], aps["o"],
    )

    dram = ctx.enter_context(tc.tile_pool(name="dram", bufs=1, space="DRAM"))
    qT_ds = [dram.tile([D, CW], MM, name=f"qTd{c}") for c in range(NCH)]
    kT_ds = [dram.tile([D, CW], MM, name=f"kTd{c}") for c in range(KCH)]

    OH = 2
    OB = DT // OH
    D2 = D // OH
    NST = min(2, DT // 2) or 1  # spill-write batch (o-tiles per DMA)

    wpool = ctx.enter_context(tc.tile_pool(name="wres", bufs=3))
    xpool = ctx.enter_context(tc.tile_pool(name="xstream", bufs=2))
    stage = ctx.enter_context(tc.tile_pool(name="stageA", bufs=2))
    ps512 = ctx.enter_context(tc.tile_pool(name="ps512", bufs=3, space="PSUM"))
    psO = ctx.enter_context(tc.tile_pool(name="psO", bufs=2, space="PSUM"))
    psD = ctx.enter_context(tc.tile_pool(name="psD", bufs=1, space="PSUM"))
    cpool = ctx.enter_context(tc.tile_pool(name="consts", bufs=1))
    vpool = ctx.enter_context(tc.tile_pool(name="vres", bufs=1))
    qpool = ctx.enter_context(tc.tile_pool(name="qc", bufs=1))
    ppool = ctx.enter_context(tc.tile_pool(name="pT", bufs=16))
    kpool = ctx.enter_context(tc.tile_pool(name="kt", bufs=3))
    mpool = ctx.enter_context(tc.tile_pool(name="mt", bufs=2))
    spool = ctx.enter_context(tc.tile_pool(name="stageB", bufs=1))

    ones_f = cpool.tile([P, 1], F32, tag="ones_f")
    nc.vector.memset(ones_f, 1.0)
    v_sb = vpool.tile([P, T, D], MM, tag="v")

    def x_load(xap, c, name):
        xs = xpool.tile([P, DT, CW], MM, tag="xs", name=name)
        rr = xap.rearrange("(dt p) n -> p dt n", p=P)[:, :, c * CW:(c + 1) * CW]
        hd = DT // 2
        nc.sync.dma_start(xs[:, :hd, :], rr[:, :hd, :])
        nc.sync.dma_start(xs[:, hd:, :], rr[:, hd:, :])
        return xs

    def w_load(wT, h, name):
        wh = wpool.tile([P, DT, D2], MM, tag="w", name=name)
        half = D2 // 2
        rr = wT.rearrange("(dt p) o -> p dt o", p=P)
        nc.sync.dma_start(wh[:, :, :half], rr[:, :, h * D2:h * D2 + half])
        nc.sync.dma_start(wh[:, :, half:], rr[:, :, h * D2 + half:(h + 1) * D2])
        return wh

    # ---------------- Phase A: projections ----------------
    # Q^T -> qT_ds[c]
    # emission order tuned for the in-order DMA queue: first W half, then
    # the first x chunk, then the remaining W halves and prefetched x
    wqs = []
    rrq = wqT.rearrange("(dt p) o -> p dt o", p=P)
    for h in range(OH):
        wqs.append(wpool.tile([P, DT, D2], MM, tag="w", name=f"wq{h}"))
    half = D2 // 2
    nc.sync.dma_start(wqs[0][:, :, :half], rrq[:, :, :half])
    xqs = []
    for c in range(NCH):
        xqs.append(xpool.tile([P, DT, CW], MM, tag="xs", name=f"xq{c}"))
        rr = xTq.rearrange("(dt p) n -> p dt n", p=P)[:, :, c * CW:(c + 1) * CW]
        hd = DT // 2
        nc.sync.dma_start(xqs[c][:, :hd, :], rr[:, :hd, :])
        nc.sync.dma_start(xqs[c][:, hd:, :], rr[:, hd:, :])
        if c == 0:
            nc.sync.dma_start(wqs[0][:, :, half:], rrq[:, :, half:D2])
            for h in range(1, OH):
                nc.sync.dma_start(wqs[h], rrq[:, :, h * D2:(h + 1) * D2])
    for c in range(NCH):
        xq = xqs[c]
        for h in range(OH):
            for og in range(OB // NST):
                st = stage.tile([P, NST, CW], MM, tag="stA", name="stq")
                for j in range(NST):
                    o4 = og * NST + j
                    ps = ps512.tile([P, CW], F32, tag="ps512", name="psq")
                    for dt in range(DT):
                        nc.tensor.matmul(
                            ps, wqs[h][:, dt, o4 * P:(o4 + 1) * P], xq[:, dt, :],
                            start=(dt == 0), stop=(dt == DT - 1))
                    nc.vector.tensor_copy(st[:, j, :], ps)
                o0 = h * OB + og * NST
                nc.sync.dma_start(
                    qT_ds[c].rearrange("(o p) w -> p o w", p=P)[:, o0:o0 + NST, :],
                    st)

    # K^T -> kT_ds[c]
    wks = None
    for c in range(KCH):
        xk = x_load(xT, c, f"xk{c}")
        if wks is None:
            wks = [w_load(wkT, h, f"wk{h}") for h in range(OH)]
        for h in range(OH):
            for og in range(OB // NST):
                st = stage.tile([P, NST, CW], MM, tag="stA", name="stk")
                for j in range(NST):
                    o4 = og * NST + j
                    ps = ps512.tile([P, CW], F32, tag="ps512", name="psk")
                    for dt in range(DT):
                        nc.tensor.matmul(
                            ps, wks[h][:, dt, o4 * P:(o4 + 1) * P], xk[:, dt, :],
                            start=(dt == 0), stop=(dt == DT - 1))
                    nc.vector.tensor_copy(st[:, j, :], ps)
                o0 = h * OB + og * NST
                nc.sync.dma_start(
                    kT_ds[c].rearrange("(o p) w -> p o w", p=P)[:, o0:o0 + NST, :],
                    st)

    # V -> resident v_sb; x read once, both wv halves held
    wvs = None
    for c in range(KCH):
        xk = x_load(xT, c, f"xkv{c}")
        if wvs is None:
            wvs = []
            for oc in range(OCH):
                wvh = wpool.tile([P, DT, OCW], MM, tag="w", name=f"wv{oc}")
                nc.sync.dma_start(
                    wvh,
                    wvT.rearrange("(dt p) o -> p dt o", p=P)
                    [:, :, oc * OCW:(oc + 1) * OCW])
                wvs.append(wvh)
        for kb in range(KB):
            for oc in range(OCH):
                ps = ps512.tile([P, OCW], F32, tag="ps512", name="psv")
                for dt in range(DT):
                    nc.tensor.matmul(
                        ps, xk[:, dt, kb * P:(kb + 1) * P], wvs[oc][:, dt, :],
                        start=(dt == 0), stop=(dt == DT - 1))
                nc.vector.tensor_copy(
                    v_sb[:, c * KB + kb, oc * OCW:(oc + 1) * OCW], ps)

    # ---------------- Phase B: attention ----------------
    mask_idx = 0
    for c in range(NCH):
        qc = qpool.tile([P, DT, CW], MM, tag="qc", name=f"qc{c}")
        nc.scalar.dma_start(qc, qT_ds[c].rearrange("(dt p) w -> p dt w", p=P))

        pTs = []
        for t in range(cfg.st_ext[c]):
            kt = kpool.tile([P, DT, P], MM, tag="kt")
            kc, ko = t // KB, (t % KB) * P
            nc.sync.dma_start(
                kt, kT_ds[kc].rearrange("(dt p) w -> p dt w", p=P)[:, :, ko:ko + P])
            ps = ps512.tile([P, CW], F32, tag="ps512", name="pss")
            for dt in range(DT):
                nc.tensor.matmul(
                    ps, kt[:, dt, :], qc[:, dt, :],
                    start=(dt == 0), stop=(dt == DT - 1))
            pT = ppool.tile([P, CW], MM, tag="pT", name=f"pT_{c}_{t}")
            nc.scalar.activation(
                pT, ps, mybir.ActivationFunctionType.Exp, scale=cfg.scale)
            mt = mpool.tile([P, CW], U8, tag="mt")
            nc.sync.dma_start(mt, mask[mask_idx])
            mask_idx += 1
            nc.vector.tensor_mul(pT, pT, mt)
            pTs.append(pT)

        psd = psD.tile([P, KB], F32, tag="psD", name=f"psd{c}")
        for b in range(KB):
            pso = psO.tile([P, D], F32, tag="psO")
            E = cfg.av_ext[c][b]
            for t in range(E):
                lh = pTs[t][:, b * P:(b + 1) * P]
                for oc in range(OCH):
                    nc.tensor.matmul(
                        pso[:, oc * OCW:(oc + 1) * OCW], lh,
                        v_sb[:, t, oc * OCW:(oc + 1) * OCW],
                        start=(t == 0), stop=(t == E - 1))
            for t in range(E):
                lh = pTs[t][:, b * P:(b + 1) * P]
                nc.tensor.matmul(
                    psd[:, b:b + 1], lh.bitcast(F32), ones_f,
                    start=(t == 0), stop=(t == E - 1))
            rcp = spool.tile([P, 1], F32, tag="rcp")
            nc.vector.reciprocal(rcp, psd[:, b:b + 1])
            osb = spool.tile([P, D], F32, tag="osb")
            nc.scalar.activation(
                osb, pso, mybir.ActivationFunctionType.Copy,
                scale=rcp[:, 0:1])
            nc.scalar.dma_start(o_ap[c * CW + b * P:c * CW + (b + 1) * P, :], osb)


def build_program(cfg: Cfg):
    nc = bacc.Bacc("TRN2", dynamic_dma_scratch_size=2048)
    aps = {
        "xT": nc.dram_tensor("xT", [cfg.D, cfg.SEQ], MM, kind="ExternalInput").ap(),
        "xTq": nc.dram_tensor("xTq", [cfg.D, cfg.R], MM, kind="ExternalInput").ap(),
        "wqT": nc.dram_tensor("wqT", [cfg.D, cfg.D], MM, kind="ExternalInput").ap(),
        "wkT": nc.dram_tensor("wkT", [cfg.D, cfg.D], MM, kind="ExternalInput").ap(),
        "wvT": nc.dram_tensor("wvT", [cfg.D, cfg.D], MM, kind="ExternalInput").ap(),
        "mask": nc.dram_tensor(
            "mask", [cfg.n_mask_tiles, P, cfg.CW], U8, kind="ExternalInput").ap(),
        "o": nc.dram_tensor("o", [cfg.R, cfg.D], F32, kind="ExternalOutput").ap(),
    }
    with tile.TileContext(nc) as tc:
        with ExitStack() as ctx:
            _emit(ctx, tc, cfg, aps)
    nc.compile()
    return nc


def make_mask(cfg: Cfg, qglob: np.ndarray) -> np.ndarray:
    """u8 mask tiles: 1 where k_global > q_global (entry masked out)."""
    m = np.zeros((cfg.n_mask_tiles, P, cfg.CW), dtype=np.uint8)
    idx = 0
    for c in range(cfg.NCH):
        qg = qglob[c * cfg.CW:(c + 1) * cfg.CW]  # [CW]
        for t in range(cfg.st_ext[c]):
            kg = np.arange(t * P, (t + 1) * P)  # [P]
            m[idx] = (kg[:, None] <= qg[None, :]).astype(np.uint8)
            idx += 1
    return m


def make_core_inputs(cfg: Cfg, x_b: np.ndarray, wqT, wkT, wvT, m: int):
    blocks = q_blocks(cfg, m)
    qglob = np.concatenate([np.arange(b * P, (b + 1) * P) for b in blocks])
    xT_b = np.ascontiguousarray(x_b.T)
    return {
        "xT": xT_b,
        "xTq": np.ascontiguousarray(xT_b[:, qglob]),
        "wqT": wqT,
        "wkT": wkT,
        "wvT": wvT,
        "mask": make_mask(cfg, qglob),
    }, qglob


_prog_cache = {}


def get_program(cfg: Cfg):
    if cfg not in _prog_cache:
        _prog_cache[cfg] = build_program(cfg)
    return _prog_cache[cfg]


def run(x, W_query, W_key, W_value, trace=False, trace_cores=None):
    """Returns (out [B, N, D], BassKernelResults)."""
    cfg = real_cfg()
    B = x.shape[0]
    nc = get_program(cfg)
    wqT = np.ascontiguousarray(np.asarray(W_query, dtype=np.float32).T)
    wkT = np.ascontiguousarray(np.asarray(W_key, dtype=np.float32).T)
    wvT = np.ascontiguousarray(np.asarray(W_value, dtype=np.float32).T)
    x = np.asarray(x, dtype=np.float32)

    in_maps = []
    qglobs = []
    for core in range(2 * B):
        b, m = core // 2, core % 2
        im, qglob = make_core_inputs(cfg, x[b], wqT, wkT, wvT, m)
        in_maps.append(im)
        qglobs.append(qglob)

    res = run_bass_kernel_spmd(
        nc, in_maps, list(range(2 * B)), trace=trace,
        trace_cores=trace_cores)

    out = np.empty((B, cfg.SEQ, cfg.D), dtype=np.float32)
    for core in range(2 * B):
        b = core // 2
        out[b][qglobs[core]] = res.results[core]["o"]
    return out, res


def kernel(**inputs) -> np.ndarray:
    out, _ = run(
        inputs["x"], inputs["W_query"], inputs["W_key"], inputs["W_value"])
    return out
